# revision 33
# baseline (speedup 1.0000x reference)
"""Trainium2 Bass kernel for a char-CNN (embed lookup + conv1d(K=5,pad=2) + bias + maxpool).

Math: out[n, f] = max_w ( b[f] + sum_k sum_d  E[ids[n, w+k-2], d] * Wc[f, d, k] )

Strategy (pure data-parallel over 8 cores, 4096 tokens each):
  * Host-side constant folding (weights only): G[k][v, f] = sum_d E[v, d] * Wc[f, d, k].
    The embedding+conv collapses to y[n,:,w] = sum_k G[k][ids[n,w+k-2], :] + b.
  * On device, table lookup is done on the TensorEngine as one-hot matmuls with
    contraction over the vocab (96) plus a constant-ones row that carries the bias:
      - broadcast ids across partitions with a K=1 ones-matmul  -> psum [96, cols]
      - one-hot = is_equal(bcast, iota_per_partition) on the VectorE -> fp16 [97, cols]
      - 5 PSUM-accumulated matmuls against G_k (fp16), shifted along the
        char-position axis with per-tap restricted ranges (conv boundary handling)
      - reduce_max over the 16 positions on VectorE
  * Output is produced as [F, n_shard] per core; host transposes/concats shards.
"""

import numpy as np

import concourse.bass as bass
import concourse.bacc as bacc
import concourse.mybir as mybir
from concourse.tile import TileContext
from concourse.bass_utils import run_bass_kernel_spmd

# Problem shapes (hardcoded per contract)
N, W = 32768, 16
VOCAB, D, F, K = 96, 100, 100, 5
N_CORES = 8
NSH = N // N_CORES            # tokens per core = 4096
UNIT = 64                     # tokens per pipeline unit (=> 1024 one-hot cols)
NUNIT = NSH // UNIT           # 64
GROUP = 512                   # tokens per ids DMA
NGROUP = NSH // GROUP         # 8
UPG = GROUP // UNIT           # units per group = 8
VP = VOCAB + 1                # 96 vocab rows + 1 ones row (bias)
CW = K * F + 4 + VOCAB        # packed consts width: 500 gtab + pad + iota/ones

f16 = mybir.dt.float16
f32 = mybir.dt.float32
f32r = mybir.dt.float32r
i32 = mybir.dt.int32


def _r(ap):
    # fp32 "raw" streaming mode: full precision, 1 col/cycle on PE (vs 4 for f32)
    return ap.bitcast(f32r)


def build_nc():
    nc = bacc.Bacc("TRN2", target_bir_lowering=False)

    ids_d = nc.dram_tensor("ids", [NSH, W], i32, kind="ExternalInput")
    # G split tables: [v, (split s, tap k), f]  s=0 -> fp16(G), s=1 -> fp16(G - hi)
    gtab_d = nc.dram_tensor("gtab", [VP, 2 * K, F], f32r, kind="ExternalInput")
    iota_d = nc.dram_tensor("iota", [VOCAB, 1], f32, kind="ExternalInput")
    ones_d = nc.dram_tensor("ones", [1, VOCAB], f32r, kind="ExternalInput")
    oinit_d = nc.dram_tensor("oinit", [VP, W + 4, UNIT], f32r, kind="ExternalInput")
    out_d = nc.dram_tensor("out", [NGROUP, F, GROUP], f32, kind="ExternalOutput")

    with TileContext(nc) as tc:
        with (
            tc.tile_pool(name="consts", bufs=1) as consts,
            tc.tile_pool(name="outp", bufs=2) as outp,
            tc.tile_pool(name="idsp", bufs=3) as idsp,
            tc.tile_pool(name="psA", bufs=2, space="PSUM") as psA,
            tc.tile_pool(name="psB", bufs=2, space="PSUM") as psB,
        ):
            gtab = consts.tile([VP, 2 * K, F], f32r)
            nc.sync.dma_start(
                out=gtab.rearrange("v s f -> v (s f)"),
                in_=gtab_d.rearrange("v s f -> v (s f)"),
            )
            iota_t = consts.tile([VOCAB, 1], f32)
            nc.sync.dma_start(out=iota_t, in_=iota_d[:, :])
            ones_t = consts.tile([1, VOCAB], f32r)
            nc.sync.dma_start(out=ones_t, in_=ones_d[:, :])



            # Two persistent one-hot tiles, padded layout [VP, UNIT, W+4]:
            # char position w at column w+2, pad columns {0,1,18,19} stay zero,
            # row 96 constant 1.0 (bias row, consumed only by center tap).
            o_tiles = []
            for j in range(2):
                ot = consts.tile([VP, W + 4, UNIT], f32r, tag=f"onehot{j}")
                nc.sync.dma_start(
                    out=ot.rearrange("v p t -> v (p t)"),
                    in_=oinit_d.rearrange("v p t -> v (p t)"),
                )
                o_tiles.append(ot)

            ids_tiles = {}

            def load_ids(g):
                idst = idsp.tile([1, GROUP * W], f32r, tag="ids")
                nc.gpsimd.dma_start(
                    out=idst[:, :],
                    in_=ids_d[g * GROUP : (g + 1) * GROUP, :],
                )
                ids_tiles[g] = idst

            def bcast(u):
                # broadcast ids across 96 partitions (K=1 matmul) + one-hot
                g, uu = divmod(u, UPG)
                idst = ids_tiles[g]
                bc = psA.tile([VOCAB, UNIT, W], f32, tag="bcast")
                for h in range(2):
                    c0 = (uu * UNIT + h * 32) * W
                    nc.tensor.matmul(
                        bc[:, h * 32 : (h + 1) * 32, :],
                        ones_t[:, :],
                        idst[0:1, c0 : c0 + 512],
                        start=True,
                        stop=True,
                    )
                # one-hot: O[v, t, w+2] = (ids[t, w] == v)
                o_t = o_tiles[u % 2]
                nc.vector.tensor_scalar(
                    out=o_t[0:VOCAB, 2 : 2 + W, :].rearrange("v p t -> v t p"),
                    in0=bc[:, :, :],
                    scalar1=iota_t[:, 0:1],
                    scalar2=None,
                    op0=mybir.AluOpType.is_equal,
                )

            # PE warmup: tiny matmuls keep the HAM activity window busy while
            # the init DMAs land, so real matmuls start at full clock.
            warm = psA.tile([1, 1], f32, tag="bcast")
            for _ in range(48):
                nc.tensor.matmul(
                    warm[0:1, 0:1],
                    iota_t[0:1, 0:1],
                    iota_t[0:1, 0:1],
                    start=True,
                    stop=True,
                )

            load_ids(0)
            load_ids(1)
            bcast(0)
            out_sb = None
            for u in range(NUNIT):
                g, uu = divmod(u, UPG)
                if uu == 0:
                    out_sb = outp.tile([F, GROUP], f32, tag="osb")
                    if g + 2 < NGROUP:
                        load_ids(g + 2)
                # emit next unit's bcast+one-hot BEFORE this unit's taps so the
                # in-order PE queue never stalls waiting on the DVE is_equal.
                if u + 1 < NUNIT:
                    bcast(u + 1)

                o_t = o_tiles[u % 2]
                # 5 taps x 2 precision splits, PSUM-accumulated (N=512 each)
                ys = [psB.tile([F, W, 32], f32, tag=f"y{h}", name=f"y{h}") for h in range(2)]
                first = True
                for s in range(2):
                    for k in range(K):
                        for h in range(2):
                            nc.tensor.matmul(
                                ys[h][:, :, :],
                                gtab[:, s * K + k, :],
                                o_t[:, k : k + W, h * 32 : (h + 1) * 32],
                                start=first,
                                stop=(s == 1 and k == K - 1),
                                skip_group_check=True,
                            )
                        first = False

                # max over the 16 char positions
                for h in range(2):
                    nc.vector.reduce_max(
                        out=out_sb[:, uu * UNIT + h * 32 : uu * UNIT + (h + 1) * 32],
                        in_=ys[h].rearrange("f w t -> f t w"),
                        axis=mybir.AxisListType.X,
                    )

                if uu == UPG - 1:
                    # stream this group's result out to DRAM (contiguous block)
                    nc.sync.dma_start(out=out_d[g, :, :], in_=out_sb[:, :])

    nc.compile()
    return nc


def _round_f32r(x):
    """FP32R keeps 11 explicit mantissa bits (low 12 bits of fp32 zeroed)."""
    b = np.asarray(x, np.float32).view(np.uint32)
    b = (b + 0x800) & np.uint32(0xFFFFF000)
    return b.view(np.float32)


def make_consts(embed_table, conv_w, conv_b):
    # G[k][v, f] = sum_d E[v, d] * Wc[f, d, k] in float64, split hi/lo f32r
    G = np.einsum(
        "vd,fdk->kvf", embed_table.astype(np.float64), conv_w.astype(np.float64)
    )
    Gf = np.zeros((K, VP, F), np.float64)
    Gf[:, 0:VOCAB, :] = G
    Gf[2, VOCAB, :] = conv_b.astype(np.float64)  # bias rides center tap
    hi = _round_f32r(Gf.astype(np.float32))
    lo = _round_f32r((Gf - hi.astype(np.float64)).astype(np.float32))
    gtab = np.zeros((VP, 2 * K, F), np.float32)
    gtab[:, 0:K, :] = np.transpose(hi, (1, 0, 2))
    gtab[:, K : 2 * K, :] = np.transpose(lo, (1, 0, 2))
    iota = np.arange(VOCAB, dtype=np.float32).reshape(VOCAB, 1)
    ones = np.ones((1, VOCAB), np.float32)
    oinit = np.zeros((VP, W + 4, UNIT), np.float32)
    oinit[VOCAB, :, :] = 1.0
    return gtab, iota, ones, oinit


_NC_CACHE = {}

# Test-harness knobs (ignored by normal kernel() use)
TRACE = False
LAST_RESULT = None


def kernel(char_ids, embed_table, conv_w, conv_b):
    global LAST_RESULT
    char_ids = np.asarray(char_ids)
    gtab, iota, ones, oinit = make_consts(
        np.asarray(embed_table), np.asarray(conv_w), np.asarray(conv_b)
    )

    if "nc" not in _NC_CACHE:
        _NC_CACHE["nc"] = build_nc()
    nc = _NC_CACHE["nc"]

    in_maps = []
    for c in range(N_CORES):
        shard = np.ascontiguousarray(char_ids[c * NSH : (c + 1) * NSH])
        in_maps.append(
            {"ids": shard, "gtab": gtab, "iota": iota, "ones": ones, "oinit": oinit}
        )

    kwargs = {}
    if TRACE:
        kwargs = dict(trace=True, trace_cores=list(range(N_CORES)))
    res = run_bass_kernel_spmd(nc, in_maps, core_ids=list(range(N_CORES)), **kwargs)
    LAST_RESULT = res

    out = np.empty((N, F), np.float32)
    for c in range(N_CORES):
        o = res.results[c]["out"]  # [NGROUP, F, GROUP]
        out[c * NSH : (c + 1) * NSH] = o.transpose(0, 2, 1).reshape(NSH, F)
    return out


# revision 34
# speedup vs baseline: 1.0941x; 1.0941x over previous
"""Trainium2 Bass kernel for a char-CNN (embed lookup + conv1d(K=5,pad=2) + bias + maxpool).

Math: out[n, f] = max_w ( b[f] + sum_k sum_d  E[ids[n, w+k-2], d] * Wc[f, d, k] )

Strategy (pure data-parallel over 8 cores, 4096 tokens each):
  * Host-side constant folding (weights only): G[k][v, f] = sum_d E[v, d] * Wc[f, d, k].
    The embedding+conv collapses to y[n,:,w] = sum_k G[k][ids[n,w+k-2], :] + b.
  * On device, table lookup is done on the TensorEngine as one-hot matmuls with
    contraction over the vocab (96) plus a constant-ones row that carries the bias:
      - broadcast ids across partitions with a K=1 ones-matmul  -> psum [96, cols]
      - one-hot = is_equal(bcast, iota_per_partition) on the VectorE -> fp16 [97, cols]
      - 5 PSUM-accumulated matmuls against G_k (fp16), shifted along the
        char-position axis with per-tap restricted ranges (conv boundary handling)
      - reduce_max over the 16 positions on VectorE
  * Output is produced as [F, n_shard] per core; host transposes/concats shards.
"""

import numpy as np

import concourse.bass as bass
import concourse.bacc as bacc
import concourse.mybir as mybir
from concourse.tile import TileContext
from concourse.bass_utils import run_bass_kernel_spmd

# Problem shapes (hardcoded per contract)
N, W = 32768, 16
VOCAB, D, F, K = 96, 100, 100, 5
N_CORES = 8
NSH = N // N_CORES            # tokens per core = 4096
UNIT = 64                     # tokens per pipeline unit (=> 1024 one-hot cols)
NUNIT = NSH // UNIT           # 64
GROUP = 512                   # tokens per ids DMA
NGROUP = NSH // GROUP         # 8
UPG = GROUP // UNIT           # units per group = 8
VP = VOCAB + 1                # 96 vocab rows + 1 ones row (bias)
CW = K * F + 4 + VOCAB        # packed consts width: 500 gtab + pad + iota/ones

f16 = mybir.dt.float16
f32 = mybir.dt.float32
f32r = mybir.dt.float32r
i32 = mybir.dt.int32


def _r(ap):
    # fp32 "raw" streaming mode: full precision, 1 col/cycle on PE (vs 4 for f32)
    return ap.bitcast(f32r)


def build_nc():
    nc = bacc.Bacc("TRN2", target_bir_lowering=False)

    ids_d = nc.dram_tensor("ids", [NSH, W], i32, kind="ExternalInput")
    # G split tables: [v, (split s, tap k), f]  s=0 -> fp16(G), s=1 -> fp16(G - hi)
    gtab_d = nc.dram_tensor("gtab", [VP, 2 * K, F], f32r, kind="ExternalInput")
    iota_d = nc.dram_tensor("iota", [VOCAB, 1], f32, kind="ExternalInput")
    ones_d = nc.dram_tensor("ones", [1, VOCAB], f32r, kind="ExternalInput")
    opad_d = nc.dram_tensor("opad", [VP, 2, UNIT], f32r, kind="ExternalInput")
    oones_d = nc.dram_tensor("oones", [1, W * UNIT], f32r, kind="ExternalInput")
    out_d = nc.dram_tensor("out", [NGROUP, F, GROUP], f32, kind="ExternalOutput")

    with TileContext(nc) as tc:
        with (
            tc.tile_pool(name="consts", bufs=1) as consts,
            tc.tile_pool(name="outp", bufs=2) as outp,
            tc.tile_pool(name="idsp", bufs=3) as idsp,
            tc.tile_pool(name="psA", bufs=2, space="PSUM") as psA,
            tc.tile_pool(name="psB", bufs=2, space="PSUM") as psB,
        ):
            gtab = consts.tile([VP, 2 * K, F], f32r)
            nc.sync.dma_start(
                out=gtab.rearrange("v s f -> v (s f)"),
                in_=gtab_d.rearrange("v s f -> v (s f)"),
            )
            iota_t = consts.tile([VOCAB, 1], f32)
            nc.sync.dma_start(out=iota_t, in_=iota_d[:, :])
            ones_t = consts.tile([1, VOCAB], f32r)
            nc.sync.dma_start(out=ones_t, in_=ones_d[:, :])



            # Two persistent one-hot tiles, padded layout [VP, UNIT, W+4]:
            # char position w at column w+2, pad columns {0,1,18,19} stay zero,
            # row 96 constant 1.0 (bias row, consumed only by center tap).
            o_tiles = []
            for j in range(2):
                ot = consts.tile([VP, W + 4, UNIT], f32r, tag=f"onehot{j}")
                # init only what is_equal never rewrites: pad columns + bias row
                nc.sync.dma_start(
                    out=ot[:, 0:2, :].rearrange("v p t -> v (p t)"),
                    in_=opad_d.rearrange("v p t -> v (p t)"),
                )
                nc.sync.dma_start(
                    out=ot[:, W + 2 : W + 4, :].rearrange("v p t -> v (p t)"),
                    in_=opad_d.rearrange("v p t -> v (p t)"),
                )
                nc.sync.dma_start(
                    out=ot[VOCAB : VOCAB + 1, 2 : 2 + W, :].rearrange("v p t -> v (p t)"),
                    in_=oones_d[:, :],
                )
                o_tiles.append(ot)

            ids_tiles = {}

            def load_ids(g):
                idst = idsp.tile([1, GROUP * W], f32r, tag="ids")
                nc.gpsimd.dma_start(
                    out=idst[:, :],
                    in_=ids_d[g * GROUP : (g + 1) * GROUP, :],
                )
                ids_tiles[g] = idst

            def bcast(u):
                # broadcast ids across 96 partitions (K=1 matmul) + one-hot
                g, uu = divmod(u, UPG)
                idst = ids_tiles[g]
                bc = psA.tile([VOCAB, UNIT, W], f32, tag="bcast")
                for h in range(2):
                    c0 = (uu * UNIT + h * 32) * W
                    nc.tensor.matmul(
                        bc[:, h * 32 : (h + 1) * 32, :],
                        ones_t[:, :],
                        idst[0:1, c0 : c0 + 512],
                        start=True,
                        stop=True,
                    )
                # one-hot: O[v, t, w+2] = (ids[t, w] == v)
                o_t = o_tiles[u % 2]
                nc.vector.tensor_scalar(
                    out=o_t[0:VOCAB, 2 : 2 + W, :].rearrange("v p t -> v t p"),
                    in0=bc[:, :, :],
                    scalar1=iota_t[:, 0:1],
                    scalar2=None,
                    op0=mybir.AluOpType.is_equal,
                )

            # PE warmup: tiny matmuls keep the HAM activity window busy while
            # the init DMAs land, so real matmuls start at full clock.
            warm = psA.tile([1, 1], f32, tag="bcast")
            for _ in range(48):
                nc.tensor.matmul(
                    warm[0:1, 0:1],
                    iota_t[0:1, 0:1],
                    iota_t[0:1, 0:1],
                    start=True,
                    stop=True,
                )

            load_ids(0)
            load_ids(1)
            bcast(0)
            out_sb = None
            for u in range(NUNIT):
                g, uu = divmod(u, UPG)
                if uu == 0:
                    out_sb = outp.tile([F, GROUP], f32, tag="osb")
                    if g + 2 < NGROUP:
                        load_ids(g + 2)
                # emit next unit's bcast+one-hot BEFORE this unit's taps so the
                # in-order PE queue never stalls waiting on the DVE is_equal.
                if u + 1 < NUNIT:
                    bcast(u + 1)

                o_t = o_tiles[u % 2]
                # 5 taps x 2 precision splits, PSUM-accumulated (N=512 each)
                ys = [psB.tile([F, W, 32], f32, tag=f"y{h}", name=f"y{h}") for h in range(2)]
                first = True
                for s in range(2):
                    for k in range(K):
                        for h in range(2):
                            nc.tensor.matmul(
                                ys[h][:, :, :],
                                gtab[:, s * K + k, :],
                                o_t[:, k : k + W, h * 32 : (h + 1) * 32],
                                start=first,
                                stop=(s == 1 and k == K - 1),
                                skip_group_check=True,
                            )
                        first = False

                # max over the 16 char positions
                for h in range(2):
                    nc.vector.reduce_max(
                        out=out_sb[:, uu * UNIT + h * 32 : uu * UNIT + (h + 1) * 32],
                        in_=ys[h].rearrange("f w t -> f t w"),
                        axis=mybir.AxisListType.X,
                    )

                if uu == UPG - 1:
                    # stream this group's result out to DRAM (contiguous block)
                    nc.sync.dma_start(out=out_d[g, :, :], in_=out_sb[:, :])

    nc.compile()
    return nc


def _round_f32r(x):
    """FP32R keeps 11 explicit mantissa bits (low 12 bits of fp32 zeroed)."""
    b = np.asarray(x, np.float32).view(np.uint32)
    b = (b + 0x800) & np.uint32(0xFFFFF000)
    return b.view(np.float32)


def make_consts(embed_table, conv_w, conv_b):
    # G[k][v, f] = sum_d E[v, d] * Wc[f, d, k] in float64, split hi/lo f32r
    G = np.einsum(
        "vd,fdk->kvf", embed_table.astype(np.float64), conv_w.astype(np.float64)
    )
    Gf = np.zeros((K, VP, F), np.float64)
    Gf[:, 0:VOCAB, :] = G
    Gf[2, VOCAB, :] = conv_b.astype(np.float64)  # bias rides center tap
    hi = _round_f32r(Gf.astype(np.float32))
    lo = _round_f32r((Gf - hi.astype(np.float64)).astype(np.float32))
    gtab = np.zeros((VP, 2 * K, F), np.float32)
    gtab[:, 0:K, :] = np.transpose(hi, (1, 0, 2))
    gtab[:, K : 2 * K, :] = np.transpose(lo, (1, 0, 2))
    iota = np.arange(VOCAB, dtype=np.float32).reshape(VOCAB, 1)
    ones = np.ones((1, VOCAB), np.float32)
    opad = np.zeros((VP, 2, UNIT), np.float32)
    opad[VOCAB, :, :] = 1.0
    oones = np.ones((1, W * UNIT), np.float32)
    return gtab, iota, ones, opad, oones


_NC_CACHE = {}

# Test-harness knobs (ignored by normal kernel() use)
TRACE = False
LAST_RESULT = None


def kernel(char_ids, embed_table, conv_w, conv_b):
    global LAST_RESULT
    char_ids = np.asarray(char_ids)
    gtab, iota, ones, opad, oones = make_consts(
        np.asarray(embed_table), np.asarray(conv_w), np.asarray(conv_b)
    )

    if "nc" not in _NC_CACHE:
        _NC_CACHE["nc"] = build_nc()
    nc = _NC_CACHE["nc"]

    in_maps = []
    for c in range(N_CORES):
        shard = np.ascontiguousarray(char_ids[c * NSH : (c + 1) * NSH])
        in_maps.append(
            {"ids": shard, "gtab": gtab, "iota": iota, "ones": ones,
             "opad": opad, "oones": oones}
        )

    kwargs = {}
    if TRACE:
        kwargs = dict(trace=True, trace_cores=list(range(N_CORES)))
    res = run_bass_kernel_spmd(nc, in_maps, core_ids=list(range(N_CORES)), **kwargs)
    LAST_RESULT = res

    out = np.empty((N, F), np.float32)
    for c in range(N_CORES):
        o = res.results[c]["out"]  # [NGROUP, F, GROUP]
        out[c * NSH : (c + 1) * NSH] = o.transpose(0, 2, 1).reshape(NSH, F)
    return out


# revision 37
# speedup vs baseline: 1.1259x; 1.0291x over previous
"""Trainium2 Bass kernel for a char-CNN (embed lookup + conv1d(K=5,pad=2) + bias + maxpool).

Math: out[n, f] = max_w ( b[f] + sum_k sum_d  E[ids[n, w+k-2], d] * Wc[f, d, k] )

Strategy (pure data-parallel over 8 cores, 4096 tokens each):
  * Host-side constant folding (weights only): G[k][v, f] = sum_d E[v, d] * Wc[f, d, k].
    The embedding+conv collapses to y[n,:,w] = sum_k G[k][ids[n,w+k-2], :] + b.
  * On device, table lookup is done on the TensorEngine as one-hot matmuls with
    contraction over the vocab (96) plus a constant-ones row that carries the bias:
      - broadcast ids across partitions with a K=1 ones-matmul  -> psum [96, cols]
      - one-hot = is_equal(bcast, iota_per_partition) on the VectorE -> fp16 [97, cols]
      - 5 PSUM-accumulated matmuls against G_k (fp16), shifted along the
        char-position axis with per-tap restricted ranges (conv boundary handling)
      - reduce_max over the 16 positions on VectorE
  * Output is produced as [F, n_shard] per core; host transposes/concats shards.
"""

import numpy as np

import concourse.bass as bass
import concourse.bacc as bacc
import concourse.mybir as mybir
from concourse.tile import TileContext
from concourse.bass_utils import run_bass_kernel_spmd

# Problem shapes (hardcoded per contract)
N, W = 32768, 16
VOCAB, D, F, K = 96, 100, 100, 5
N_CORES = 8
NSH = N // N_CORES            # tokens per core = 4096
UNIT = 64                     # tokens per pipeline unit (=> 1024 one-hot cols)
NUNIT = NSH // UNIT           # 64
GROUP = 512                   # tokens per ids DMA
NGROUP = NSH // GROUP         # 8
UPG = GROUP // UNIT           # units per group = 8
VP = VOCAB + 1                # 96 vocab rows + 1 ones row (bias)
CW = K * F + 4 + VOCAB        # packed consts width: 500 gtab + pad + iota/ones

f16 = mybir.dt.float16
f32 = mybir.dt.float32
f32r = mybir.dt.float32r
i32 = mybir.dt.int32


def _r(ap):
    # fp32 "raw" streaming mode: full precision, 1 col/cycle on PE (vs 4 for f32)
    return ap.bitcast(f32r)


def build_nc():
    nc = bacc.Bacc("TRN2", target_bir_lowering=False)

    ids_d = nc.dram_tensor("ids", [NSH, W], i32, kind="ExternalInput")
    # G split tables: [v, (split s, tap k), f]  s=0 -> fp16(G), s=1 -> fp16(G - hi)
    gtab_d = nc.dram_tensor("gtab", [VP, 2 * K, F], f32r, kind="ExternalInput")
    iota_d = nc.dram_tensor("iota", [VOCAB, 1], f32, kind="ExternalInput")
    ones_d = nc.dram_tensor("ones", [33, VOCAB], f32r, kind="ExternalInput")
    opad_d = nc.dram_tensor("opad", [VP, 2, UNIT], f32r, kind="ExternalInput")
    oones_d = nc.dram_tensor("oones", [1, W * UNIT], f32r, kind="ExternalInput")
    out_d = nc.dram_tensor("out", [NGROUP, F, GROUP], f32, kind="ExternalOutput")

    with TileContext(nc) as tc:
        with (
            tc.tile_pool(name="consts", bufs=1) as consts,
            tc.tile_pool(name="outp", bufs=2) as outp,
            tc.tile_pool(name="idsp", bufs=3) as idsp,
            tc.tile_pool(name="psA", bufs=2, space="PSUM") as psA,
            tc.tile_pool(name="psB", bufs=2, space="PSUM") as psB,
        ):
            iota_t = consts.tile([VOCAB, 1], f32)
            nc.sync.dma_start(out=iota_t, in_=iota_d[:, :])
            ones_t = consts.tile([33, VOCAB], f32r)
            nc.sync.dma_start(out=ones_t, in_=ones_d[:, :])



            # Two persistent one-hot tiles, padded layout [VP, UNIT, W+4]:
            # char position w at column w+2, pad columns {0,1,18,19} stay zero,
            # row 96 constant 1.0 (bias row, consumed only by center tap).
            o_tiles = []
            for j in range(2):
                ot = consts.tile([VP, W + 4, UNIT], f32r, tag=f"onehot{j}")
                # init only what is_equal never rewrites: pad columns + bias row
                nc.sync.dma_start(
                    out=ot[:, 0:2, :].rearrange("v p t -> v (p t)"),
                    in_=opad_d.rearrange("v p t -> v (p t)"),
                )
                nc.sync.dma_start(
                    out=ot[:, W + 2 : W + 4, :].rearrange("v p t -> v (p t)"),
                    in_=opad_d.rearrange("v p t -> v (p t)"),
                )
                nc.sync.dma_start(
                    out=ot[VOCAB : VOCAB + 1, 2 : 2 + W, :].rearrange("v p t -> v (p t)"),
                    in_=oones_d[:, :],
                )
                o_tiles.append(ot)

            ids_tiles = {}

            def load_ids(g):
                idst = idsp.tile([33, GROUP * W // 2], f32r, tag="ids")
                v = ids_d[g * GROUP : (g + 1) * GROUP, :].rearrange(
                    "(b a t) w -> b a (t w)", a=2, t=32
                )
                nc.gpsimd.dma_start(out=idst[0:1, :], in_=v[:, 0, :])
                nc.gpsimd.dma_start(out=idst[32:33, :], in_=v[:, 1, :])
                ids_tiles[g] = idst

            def bcast(u):
                # broadcast ids across 96 partitions (K=1 matmul) + one-hot
                g, uu = divmod(u, UPG)
                idst = ids_tiles[g]
                bc = psA.tile([VOCAB, UNIT, W], f32, tag="bcast")
                for h in range(2):
                    p0 = 32 * h
                    nc.tensor.matmul(
                        bc[:, h * 32 : (h + 1) * 32, :],
                        ones_t[p0 : p0 + 1, :],
                        idst[p0 : p0 + 1, uu * 512 : (uu + 1) * 512],
                        start=True,
                        stop=True,
                    )
                # one-hot: O[v, t, w+2] = (ids[t, w] == v)
                o_t = o_tiles[u % 2]
                nc.vector.tensor_scalar(
                    out=o_t[0:VOCAB, 2 : 2 + W, :].rearrange("v p t -> v t p"),
                    in0=bc[:, :, :],
                    scalar1=iota_t[:, 0:1],
                    scalar2=None,
                    op0=mybir.AluOpType.is_equal,
                )

            gtab = consts.tile([VP, 2 * K, F], f32r)
            nc.sync.dma_start(
                out=gtab.rearrange("v s f -> v (s f)"),
                in_=gtab_d.rearrange("v s f -> v (s f)"),
            )

            # PE warmup: tiny matmuls keep the HAM activity window busy while
            # the init DMAs land, so real matmuls start at full clock.
            warm = psA.tile([1, 1], f32, tag="bcast")
            for _ in range(48):
                nc.tensor.matmul(
                    warm[0:1, 0:1],
                    iota_t[0:1, 0:1],
                    iota_t[0:1, 0:1],
                    start=True,
                    stop=True,
                )

            load_ids(0)
            load_ids(1)
            bcast(0)
            out_sb = None
            for u in range(NUNIT):
                g, uu = divmod(u, UPG)
                if uu == 0:
                    out_sb = outp.tile([F, GROUP], f32, tag="osb")
                    if g + 2 < NGROUP:
                        load_ids(g + 2)
                # emit next unit's bcast+one-hot BEFORE this unit's taps so the
                # in-order PE queue never stalls waiting on the DVE is_equal.
                if u + 1 < NUNIT:
                    bcast(u + 1)

                o_t = o_tiles[u % 2]
                # 5 taps x 2 precision splits, PSUM-accumulated (N=512 each)
                ys = [psB.tile([F, W, 32], f32, tag=f"y{h}", name=f"y{h}") for h in range(2)]
                first = True
                for s in range(2):
                    for k in range(K):
                        for h in range(2):
                            nc.tensor.matmul(
                                ys[h][:, :, :],
                                gtab[:, s * K + k, :],
                                o_t[:, k : k + W, h * 32 : (h + 1) * 32],
                                start=first,
                                stop=(s == 1 and k == K - 1),
                                skip_group_check=True,
                            )
                        first = False

                # max over the 16 char positions
                for h in range(2):
                    nc.vector.reduce_max(
                        out=out_sb[:, uu * UNIT + h * 32 : uu * UNIT + (h + 1) * 32],
                        in_=ys[h].rearrange("f w t -> f t w"),
                        axis=mybir.AxisListType.X,
                    )

                if uu == UPG - 1:
                    # stream this group's result out to DRAM (contiguous block)
                    nc.sync.dma_start(out=out_d[g, :, :], in_=out_sb[:, :])

    nc.compile()
    return nc


def _round_f32r(x):
    """FP32R keeps 11 explicit mantissa bits (low 12 bits of fp32 zeroed)."""
    b = np.asarray(x, np.float32).view(np.uint32)
    b = (b + 0x800) & np.uint32(0xFFFFF000)
    return b.view(np.float32)


def make_consts(embed_table, conv_w, conv_b):
    # G[k][v, f] = sum_d E[v, d] * Wc[f, d, k] in float64, split hi/lo f32r
    G = np.einsum(
        "vd,fdk->kvf", embed_table.astype(np.float64), conv_w.astype(np.float64)
    )
    Gf = np.zeros((K, VP, F), np.float64)
    Gf[:, 0:VOCAB, :] = G
    Gf[2, VOCAB, :] = conv_b.astype(np.float64)  # bias rides center tap
    hi = _round_f32r(Gf.astype(np.float32))
    lo = _round_f32r((Gf - hi.astype(np.float64)).astype(np.float32))
    gtab = np.zeros((VP, 2 * K, F), np.float32)
    gtab[:, 0:K, :] = np.transpose(hi, (1, 0, 2))
    gtab[:, K : 2 * K, :] = np.transpose(lo, (1, 0, 2))
    iota = np.arange(VOCAB, dtype=np.float32).reshape(VOCAB, 1)
    ones = np.zeros((33, VOCAB), np.float32)
    ones[0, :] = 1.0
    ones[32, :] = 1.0
    opad = np.zeros((VP, 2, UNIT), np.float32)
    opad[VOCAB, :, :] = 1.0
    oones = np.ones((1, W * UNIT), np.float32)
    return gtab, iota, ones, opad, oones


_NC_CACHE = {}

# Test-harness knobs (ignored by normal kernel() use)
TRACE = False
LAST_RESULT = None


def kernel(char_ids, embed_table, conv_w, conv_b):
    global LAST_RESULT
    char_ids = np.asarray(char_ids)
    gtab, iota, ones, opad, oones = make_consts(
        np.asarray(embed_table), np.asarray(conv_w), np.asarray(conv_b)
    )

    if "nc" not in _NC_CACHE:
        _NC_CACHE["nc"] = build_nc()
    nc = _NC_CACHE["nc"]

    in_maps = []
    for c in range(N_CORES):
        shard = np.ascontiguousarray(char_ids[c * NSH : (c + 1) * NSH])
        in_maps.append(
            {"ids": shard, "gtab": gtab, "iota": iota, "ones": ones,
             "opad": opad, "oones": oones}
        )

    kwargs = {}
    if TRACE:
        kwargs = dict(trace=True, trace_cores=list(range(N_CORES)))
    res = run_bass_kernel_spmd(nc, in_maps, core_ids=list(range(N_CORES)), **kwargs)
    LAST_RESULT = res

    out = np.empty((N, F), np.float32)
    for c in range(N_CORES):
        o = res.results[c]["out"]  # [NGROUP, F, GROUP]
        out[c * NSH : (c + 1) * NSH] = o.transpose(0, 2, 1).reshape(NSH, F)
    return out


# revision 38
# speedup vs baseline: 1.1279x; 1.0018x over previous
"""Trainium2 Bass kernel for a char-CNN (embed lookup + conv1d(K=5,pad=2) + bias + maxpool).

Math: out[n, f] = max_w ( b[f] + sum_k sum_d  E[ids[n, w+k-2], d] * Wc[f, d, k] )

Strategy (pure data-parallel over 8 cores, 4096 tokens each):
  * Host-side constant folding (weights only): G[k][v, f] = sum_d E[v, d] * Wc[f, d, k].
    The embedding+conv collapses to y[n,:,w] = sum_k G[k][ids[n,w+k-2], :] + b.
  * On device, table lookup is done on the TensorEngine as one-hot matmuls with
    contraction over the vocab (96) plus a constant-ones row that carries the bias:
      - broadcast ids across partitions with a K=1 ones-matmul  -> psum [96, cols]
      - one-hot = is_equal(bcast, iota_per_partition) on the VectorE -> fp16 [97, cols]
      - 5 PSUM-accumulated matmuls against G_k (fp16), shifted along the
        char-position axis with per-tap restricted ranges (conv boundary handling)
      - reduce_max over the 16 positions on VectorE
  * Output is produced as [F, n_shard] per core; host transposes/concats shards.
"""

import numpy as np

import concourse.bass as bass
import concourse.bacc as bacc
import concourse.mybir as mybir
from concourse.tile import TileContext
from concourse.bass_utils import run_bass_kernel_spmd

# Problem shapes (hardcoded per contract)
N, W = 32768, 16
VOCAB, D, F, K = 96, 100, 100, 5
N_CORES = 8
NSH = N // N_CORES            # tokens per core = 4096
UNIT = 64                     # tokens per pipeline unit (=> 1024 one-hot cols)
NUNIT = NSH // UNIT           # 64
GROUP = 512                   # tokens per ids DMA
NGROUP = NSH // GROUP         # 8
UPG = GROUP // UNIT           # units per group = 8
VP = VOCAB + 1                # 96 vocab rows + 1 ones row (bias)
CW = K * F + 4 + VOCAB        # packed consts width: 500 gtab + pad + iota/ones

f16 = mybir.dt.float16
f32 = mybir.dt.float32
f32r = mybir.dt.float32r
i32 = mybir.dt.int32


def _r(ap):
    # fp32 "raw" streaming mode: full precision, 1 col/cycle on PE (vs 4 for f32)
    return ap.bitcast(f32r)


def build_nc():
    nc = bacc.Bacc("TRN2", target_bir_lowering=False)

    ids_d = nc.dram_tensor("ids", [NSH, W], i32, kind="ExternalInput")
    # G split tables: [v, (split s, tap k), f]  s=0 -> fp16(G), s=1 -> fp16(G - hi)
    gtab_d = nc.dram_tensor("gtab", [VP, 2 * K, F], f32r, kind="ExternalInput")
    iota_d = nc.dram_tensor("iota", [VOCAB, 1], f32, kind="ExternalInput")
    ones_d = nc.dram_tensor("ones", [33, VOCAB], f32r, kind="ExternalInput")
    opad_d = nc.dram_tensor("opad", [VP, 2, UNIT], f32r, kind="ExternalInput")
    oones_d = nc.dram_tensor("oones", [1, W * UNIT], f32r, kind="ExternalInput")
    out_d = nc.dram_tensor("out", [NGROUP, F, GROUP], f32, kind="ExternalOutput")

    with TileContext(nc) as tc:
        with (
            tc.tile_pool(name="consts", bufs=1) as consts,
            tc.tile_pool(name="outp", bufs=2) as outp,
            tc.tile_pool(name="idsp", bufs=3) as idsp,
            tc.tile_pool(name="psA", bufs=2, space="PSUM") as psA,
            tc.tile_pool(name="psB", bufs=2, space="PSUM") as psB,
        ):
            iota_t = consts.tile([VOCAB, 1], f32)
            nc.sync.dma_start(out=iota_t, in_=iota_d[:, :])
            # touch the DVE with the is_equal opcode early: absorbs the
            # engine's first-dispatch latency during the init phase.
            dve_warm = consts.tile([VOCAB, 1], f32, tag="dve_warm")
            nc.vector.tensor_scalar(
                out=dve_warm[:, :],
                in0=iota_t[:, :],
                scalar1=iota_t[:, 0:1],
                scalar2=None,
                op0=mybir.AluOpType.is_equal,
            )
            ones_t = consts.tile([33, VOCAB], f32r)
            nc.sync.dma_start(out=ones_t, in_=ones_d[:, :])



            # Two persistent one-hot tiles, padded layout [VP, UNIT, W+4]:
            # char position w at column w+2, pad columns {0,1,18,19} stay zero,
            # row 96 constant 1.0 (bias row, consumed only by center tap).
            o_tiles = []
            for j in range(2):
                ot = consts.tile([VP, W + 4, UNIT], f32r, tag=f"onehot{j}")
                # init only what is_equal never rewrites: pad columns + bias row
                nc.sync.dma_start(
                    out=ot[:, 0:2, :].rearrange("v p t -> v (p t)"),
                    in_=opad_d.rearrange("v p t -> v (p t)"),
                )
                nc.sync.dma_start(
                    out=ot[:, W + 2 : W + 4, :].rearrange("v p t -> v (p t)"),
                    in_=opad_d.rearrange("v p t -> v (p t)"),
                )
                nc.sync.dma_start(
                    out=ot[VOCAB : VOCAB + 1, 2 : 2 + W, :].rearrange("v p t -> v (p t)"),
                    in_=oones_d[:, :],
                )
                o_tiles.append(ot)

            ids_tiles = {}

            def load_ids(g):
                idst = idsp.tile([33, GROUP * W // 2], f32r, tag="ids")
                v = ids_d[g * GROUP : (g + 1) * GROUP, :].rearrange(
                    "(b a t) w -> b a (t w)", a=2, t=32
                )
                nc.gpsimd.dma_start(out=idst[0:1, :], in_=v[:, 0, :])
                nc.gpsimd.dma_start(out=idst[32:33, :], in_=v[:, 1, :])
                ids_tiles[g] = idst

            def bcast(u):
                # broadcast ids across 96 partitions (K=1 matmul) + one-hot
                g, uu = divmod(u, UPG)
                idst = ids_tiles[g]
                bc = psA.tile([VOCAB, UNIT, W], f32, tag="bcast")
                for h in range(2):
                    p0 = 32 * h
                    nc.tensor.matmul(
                        bc[:, h * 32 : (h + 1) * 32, :],
                        ones_t[p0 : p0 + 1, :],
                        idst[p0 : p0 + 1, uu * 512 : (uu + 1) * 512],
                        start=True,
                        stop=True,
                    )
                # one-hot: O[v, t, w+2] = (ids[t, w] == v)
                o_t = o_tiles[u % 2]
                nc.vector.tensor_scalar(
                    out=o_t[0:VOCAB, 2 : 2 + W, :].rearrange("v p t -> v t p"),
                    in0=bc[:, :, :],
                    scalar1=iota_t[:, 0:1],
                    scalar2=None,
                    op0=mybir.AluOpType.is_equal,
                )

            gtab = consts.tile([VP, 2 * K, F], f32r)
            nc.sync.dma_start(
                out=gtab.rearrange("v s f -> v (s f)"),
                in_=gtab_d.rearrange("v s f -> v (s f)"),
            )

            # PE warmup: tiny matmuls keep the HAM activity window busy while
            # the init DMAs land, so real matmuls start at full clock.
            warm = psA.tile([1, 1], f32, tag="bcast")
            for _ in range(48):
                nc.tensor.matmul(
                    warm[0:1, 0:1],
                    iota_t[0:1, 0:1],
                    iota_t[0:1, 0:1],
                    start=True,
                    stop=True,
                )

            load_ids(0)
            load_ids(1)
            bcast(0)
            out_sb = None
            for u in range(NUNIT):
                g, uu = divmod(u, UPG)
                if uu == 0:
                    out_sb = outp.tile([F, GROUP], f32, tag="osb")
                    if g + 2 < NGROUP:
                        load_ids(g + 2)
                # emit next unit's bcast+one-hot BEFORE this unit's taps so the
                # in-order PE queue never stalls waiting on the DVE is_equal.
                if u + 1 < NUNIT:
                    bcast(u + 1)

                o_t = o_tiles[u % 2]
                # 5 taps x 2 precision splits, PSUM-accumulated (N=512 each)
                ys = [psB.tile([F, W, 32], f32, tag=f"y{h}", name=f"y{h}") for h in range(2)]
                first = True
                for s in range(2):
                    for k in range(K):
                        for h in range(2):
                            nc.tensor.matmul(
                                ys[h][:, :, :],
                                gtab[:, s * K + k, :],
                                o_t[:, k : k + W, h * 32 : (h + 1) * 32],
                                start=first,
                                stop=(s == 1 and k == K - 1),
                                skip_group_check=True,
                            )
                        first = False

                # max over the 16 char positions
                for h in range(2):
                    nc.vector.reduce_max(
                        out=out_sb[:, uu * UNIT + h * 32 : uu * UNIT + (h + 1) * 32],
                        in_=ys[h].rearrange("f w t -> f t w"),
                        axis=mybir.AxisListType.X,
                    )

                if uu == UPG - 1:
                    # stream this group's result out to DRAM (contiguous block)
                    nc.sync.dma_start(out=out_d[g, :, :], in_=out_sb[:, :])

    nc.compile()
    return nc


def _round_f32r(x):
    """FP32R keeps 11 explicit mantissa bits (low 12 bits of fp32 zeroed)."""
    b = np.asarray(x, np.float32).view(np.uint32)
    b = (b + 0x800) & np.uint32(0xFFFFF000)
    return b.view(np.float32)


def make_consts(embed_table, conv_w, conv_b):
    # G[k][v, f] = sum_d E[v, d] * Wc[f, d, k] in float64, split hi/lo f32r
    G = np.einsum(
        "vd,fdk->kvf", embed_table.astype(np.float64), conv_w.astype(np.float64)
    )
    Gf = np.zeros((K, VP, F), np.float64)
    Gf[:, 0:VOCAB, :] = G
    Gf[2, VOCAB, :] = conv_b.astype(np.float64)  # bias rides center tap
    hi = _round_f32r(Gf.astype(np.float32))
    lo = _round_f32r((Gf - hi.astype(np.float64)).astype(np.float32))
    gtab = np.zeros((VP, 2 * K, F), np.float32)
    gtab[:, 0:K, :] = np.transpose(hi, (1, 0, 2))
    gtab[:, K : 2 * K, :] = np.transpose(lo, (1, 0, 2))
    iota = np.arange(VOCAB, dtype=np.float32).reshape(VOCAB, 1)
    ones = np.zeros((33, VOCAB), np.float32)
    ones[0, :] = 1.0
    ones[32, :] = 1.0
    opad = np.zeros((VP, 2, UNIT), np.float32)
    opad[VOCAB, :, :] = 1.0
    oones = np.ones((1, W * UNIT), np.float32)
    return gtab, iota, ones, opad, oones


_NC_CACHE = {}

# Test-harness knobs (ignored by normal kernel() use)
TRACE = False
LAST_RESULT = None


def kernel(char_ids, embed_table, conv_w, conv_b):
    global LAST_RESULT
    char_ids = np.asarray(char_ids)
    gtab, iota, ones, opad, oones = make_consts(
        np.asarray(embed_table), np.asarray(conv_w), np.asarray(conv_b)
    )

    if "nc" not in _NC_CACHE:
        _NC_CACHE["nc"] = build_nc()
    nc = _NC_CACHE["nc"]

    in_maps = []
    for c in range(N_CORES):
        shard = np.ascontiguousarray(char_ids[c * NSH : (c + 1) * NSH])
        in_maps.append(
            {"ids": shard, "gtab": gtab, "iota": iota, "ones": ones,
             "opad": opad, "oones": oones}
        )

    kwargs = {}
    if TRACE:
        kwargs = dict(trace=True, trace_cores=list(range(N_CORES)))
    res = run_bass_kernel_spmd(nc, in_maps, core_ids=list(range(N_CORES)), **kwargs)
    LAST_RESULT = res

    out = np.empty((N, F), np.float32)
    for c in range(N_CORES):
        o = res.results[c]["out"]  # [NGROUP, F, GROUP]
        out[c * NSH : (c + 1) * NSH] = o.transpose(0, 2, 1).reshape(NSH, F)
    return out
